# revision 1
# baseline (speedup 1.0000x reference)
"""Trainium2 Bass kernel for nn_AlignmentMatrix — fp16-wire version.

score[b,i,j] = [body_i ; pun_j ; body_i*pun_j] @ w_u
            = (body @ Bhat^T)[i,j] + s_pun[j]
where Bhat[j,d] = w3[d]*pun[j,d] + w1[d]  (folds s_cross and s_body).

The rel-err gate (2e-2) leaves an order of magnitude of headroom for
16-bit transport, and the kernel is DMA-bound (40 MB/core at 360 B/ns),
so the host casts body/pun to fp16 AND pre-transposes them to [b, D, L]
(pure layout change; all arithmetic stays on device): the device loads
AT/BT directly with fully-contiguous DMA, runs fp16 matmuls (f32 PSUM),
and stores the 32 MB/core output as fp16 (upcast on the host).
Total traffic 20.4 MB/core: ~2x faster than the f32 kernel.

s_pun[j] is added two ways to balance engines: W2K of the 8 row tiles
per batch fold it in via an extra PSUM-accumulating matmul (w2rep
stationary) and evict with a plain copy; the rest get it via a DVE
tensor_tensor add of a per-batch SP tile during PSUM->SBUF eviction.
The next batch's BH/SP are emitted mid-batch so evictions and stores
flow continuously across batch boundaries.

Sharding: data-parallel over batch across 8 NeuronCores (8 batches/core).
"""

import numpy as np

B, L, D = 64, 1024, 128
N_CORES = 8
BPC = B // N_CORES  # batches per core
P = 128
JT = 512  # matmul moving free dim

_CACHE = {}

DEFAULT_TUNE = {
    "pair_stores": True,
    "pair_loads": True,
    "store_engines": ["sync"],
    "load_engine": "gpsimd",
    "nat": 4,
    "outs": 12,
    "bht": 4,
    "mm_ps": 6,
    "sp_ps": 2,
    # it-tiles with pe_add (interleaved even slots when W2K=4) use the PE
    # w2rep accumulation + plain-copy eviction; the rest get s_pun via a
    # DVE tensor add of the SP tile at eviction.
    "w2k": 4,
    "half_ps": True,
    "copy_engines": ["act"],
    "add_engines": ["dve"],
    "bh_engine": "dve",
    "sp_engines": ["act", "act"],
    "bh_slot": 1,
    "sp_slots": [2, 3],
    "warmup": 24,
}


def _build(bpc=BPC, repeats=1, tune=None):
    from contextlib import ExitStack

    import concourse.tile as tile
    from concourse import bacc, mybir

    tune = dict(DEFAULT_TUNE if tune is None else tune)
    NAT_BUFS = tune.get("nat", 4)
    BHT_BUFS = tune.get("bht", 3)
    OUT_BUFS = tune.get("outs", 12)
    MM_PS_BUFS = tune.get("mm_ps", 3)
    SP_PS_BUFS = tune.get("sp_ps", 2)
    W2K = tune.get("w2k", 4)
    HALF_PS = tune.get("half_ps", False)

    f32 = mybir.dt.float32
    f16 = mybir.dt.float16
    Identity = mybir.ActivationFunctionType.Identity
    add_op = mybir.AluOpType.add

    nc = bacc.Bacc("TRN2", target_bir_lowering=False, debug=False, num_devices=N_CORES)

    # host-pre-transposed: body[b, d, i], pun[b, d, j]
    body = nc.dram_tensor("body", [bpc, D, L], f16, kind="ExternalInput").ap()
    pun = nc.dram_tensor("pun", [bpc, D, L], f16, kind="ExternalInput").ap()
    w_u = nc.dram_tensor("w_u", [3 * D, 1], f32, kind="ExternalInput").ap()
    PROXY = tune.pop("proxy", False)
    if PROXY:
        out = nc.dram_tensor("oscratch", [bpc, L, L], f16).ap()
        outx = nc.dram_tensor("out", [P, P], f16, kind="ExternalOutput").ap()
    else:
        out = nc.dram_tensor("out", [bpc, L, L], f16, kind="ExternalOutput").ap()

    with tile.TileContext(nc) as tc, ExitStack() as ctx:
        consts = ctx.enter_context(tc.tile_pool(name="consts", bufs=1))
        nat_pool = ctx.enter_context(tc.tile_pool(name="nat", bufs=NAT_BUFS))
        bht_pool = ctx.enter_context(tc.tile_pool(name="bht", bufs=BHT_BUFS))
        sp_pool = ctx.enter_context(tc.tile_pool(name="spp", bufs=2))
        out_pool = ctx.enter_context(tc.tile_pool(name="outs", bufs=OUT_BUFS))
        mm_ps = ctx.enter_context(
            tc.tile_pool(name="mm_ps", bufs=MM_PS_BUFS, space="PSUM")
        )
        sp_ps = (
            ctx.enter_context(tc.tile_pool(name="sp_ps", bufs=SP_PS_BUFS, space="PSUM"))
            if W2K < 8
            else None
        )

        ENG = {
            "sync": nc.sync,
            "gpsimd": nc.gpsimd,
            "scalar": nc.scalar,
            "vector": nc.vector,
        }
        DEFAULT_LOAD_ENG = ENG[tune.get("load_engine", "gpsimd")]
        STORE_ENGS = [ENG[e] for e in tune.get("store_engines", ["sync", "scalar"])]
        SG = tune.get("store_group", 2 if tune.get("pair_stores", True) else 1)
        PAIR_LOADS = tune.get("pair_loads", True)
        COPY_ENGS = list(tune.get("copy_engines", ["act"]))
        ADD_ENGS = list(tune.get("add_engines", ["dve"]))
        BH_ENG = tune.get("bh_engine", "dve")
        SP_ENGS = list(tune.get("sp_engines", ["act"]))

        def evict(eng_name, dst, src):
            if eng_name == "dve":
                nc.vector.tensor_copy(dst, src)
            elif eng_name == "pool":
                nc.gpsimd.tensor_copy(dst, src)
            else:
                nc.scalar.copy(dst, src)

        def issue_loads(b, eng=None):
            nb = 2 if PAIR_LOADS else 1
            natb = nat_pool.tile([P, nb, L], f16, tag="natb")
            natp = nat_pool.tile([P, nb, L], f16, tag="natp")
            bsl = slice(b, b + nb)
            LOAD_ENG = eng if eng is not None else DEFAULT_LOAD_ENG
            # pun first: everything (BH, w2rep matmuls) depends on BT
            LOAD_ENG.dma_start(natp[:], pun[bsl].rearrange("b2 d l -> d b2 l"))
            LOAD_ENG.dma_start(natb[:], body[bsl].rearrange("b2 d l -> d b2 l"))
            return natb, natp

        order = [b for _ in range(repeats) for b in range(bpc)]
        if PAIR_LOADS:
            assert bpc % 2 == 0
        # First loads on the HWDGE sync ring (best first-byte latency); the
        # tiny wcols load rides the gpsimd SWDGE in parallel so it doesn't
        # delay the bulk-load stream.
        hoisted = {0: issue_loads(order[0], eng=nc.sync)}
        nats = {}  # position-pair start -> (natb, natp)

        # wcols[:, k] = w_u[k*128:(k+1)*128, 0]; k=0 -> w1, 1 -> w2, 2 -> w3
        wcols = consts.tile([P, 3], f32)
        nc.gpsimd.dma_start(wcols[:], w_u.rearrange("(k p) one -> p (k one)", p=P))

        # PE p-state warmup: dummy matmuls during the initial DMA-ramp dead
        # time so the first real matmuls are costed at full clock.
        WARMUP = tune.get("warmup", 8)
        if WARMUP:
            zstat = consts.tile([P, 256], f16, tag="zstat")
            nc.vector.memset(zstat[:], 0.0)
            wd = mm_ps.tile([P, JT if HALF_PS else L], f32, tag="pmm")
            for _ in range(WARMUP):
                nc.tensor.matmul(wd[:, :256], zstat[:, :P], zstat[:],
                                 start=True, stop=True)

        # W2_rep[d, i] = w2[d] for all i (stationary operand broadcasting s_pun)
        zeros = consts.tile([P, P], f16)
        nc.vector.memset(zeros[:], 0.0)
        w2rep = consts.tile([P, P], f16)
        nc.scalar.activation(w2rep[:], zeros[:], Identity, bias=wcols[:, 1:2])
        if PROXY:
            sink = consts.tile([P, 512], f16)
            nc.vector.memset(sink[:], 0.0)

        def get_nat(pos):
            """nat tiles for the load-pair covering position pos."""
            p0 = pos - (pos % 2) if PAIR_LOADS else pos
            if p0 not in nats:
                nats[p0] = hoisted.pop(p0, None) or issue_loads(order[p0])
            natb, natp = nats[p0]
            sub = pos - p0
            return natb[:, sub, :], natp[:, sub, :]

        class BatchTiles:
            def __init__(self, pos):
                self.AT, self.BT = get_nat(pos)
                self.BH = None
                self.SP = None

        def emit_bh(tiles):
            tiles.BH = bht_pool.tile([P, L], f16)
            if BH_ENG == "dve":
                nc.vector.tensor_scalar(
                    tiles.BH[:], tiles.BT[:], wcols[:, 2:3], wcols[:, 0:1],
                    op0=mybir.AluOpType.mult, op1=add_op,
                )
            else:
                nc.scalar.activation(
                    tiles.BH[:], tiles.BT[:], Identity,
                    bias=wcols[:, 0:1], scale=wcols[:, 2:3],
                )

        def emit_sp_half(tiles, jh):
            """One [P,512] half of SP through a 1-bank psum tile."""
            if tiles.SP is None:
                tiles.SP = sp_pool.tile([P, L], f16, tag="sp")
            js = slice(jh * JT, (jh + 1) * JT)
            psp = sp_ps.tile([P, JT], f32, tag="spps")
            nc.tensor.matmul(psp[:], w2rep[:], tiles.BT[:, js], start=True, stop=True)
            evict(SP_ENGS[jh % len(SP_ENGS)], tiles.SP[:, js], psp[:])

        # next-batch preamble actions interleaved into this batch's it-slots
        BH_SLOT = tune.get("bh_slot", 2)
        SP_SLOTS = tune.get("sp_slots", [4, 5])

        def emit_stage(tiles, it):
            if it == BH_SLOT:
                emit_bh(tiles)
            if W2K < 8 and it in SP_SLOTS:
                emit_sp_half(tiles, SP_SLOTS.index(it))

        # batch 0 preamble runs un-pipelined up front
        cur = BatchTiles(0)
        emit_bh(cur)
        if W2K < 8:
            emit_sp_half(cur, 0)
            emit_sp_half(cur, 1)

        for idx, b in enumerate(order):
            nxt = BatchTiles(idx + 1) if idx + 1 < len(order) else None
            # prefetch loads one pair ahead
            if PAIR_LOADS and idx % 2 == 0 and idx + 2 < len(order):
                get_nat(idx + 2)

            n_store = 0
            n_copy = 0
            n_add = 0
            ot = None
            for it in range(8):
                # Interleave the two eviction paths so the plain copies and
                # the SP adds overlap instead of alternating per half-batch.
                if W2K == 4:
                    pe_add = it % 2 == 0
                elif W2K == 6:
                    pe_add = it not in (3, 7)
                else:
                    pe_add = it < W2K
                if SG > 1:
                    if it % SG == 0:
                        ot = out_pool.tile([P, SG, L], f16)
                    half = ot[:, it % SG, :]
                else:
                    ot = out_pool.tile([P, L], f16)
                    half = ot[:]
                pmm = None if HALF_PS else mm_ps.tile([P, L], f32, tag="pmm")
                for jh in range(2):
                    js = slice(jh * JT, (jh + 1) * JT)
                    if HALF_PS:
                        pjh = mm_ps.tile([P, JT], f32, tag="pmm")
                        pdst = pjh[:]
                    else:
                        pdst = pmm[:, js]
                    if pe_add:
                        nc.tensor.matmul(
                            pdst, w2rep[:], cur.BT[:, js], start=True, stop=False
                        )
                    nc.tensor.matmul(
                        pdst,
                        cur.AT[:, it * P : (it + 1) * P],
                        cur.BH[:, js],
                        start=not pe_add,
                        stop=True,
                    )
                    if not HALF_PS:
                        continue
                    # per-half eviction
                    if pe_add:
                        evict(COPY_ENGS[n_copy % len(COPY_ENGS)], half[:, js], pdst)
                        n_copy += 1
                    else:
                        eng = ADD_ENGS[n_add % len(ADD_ENGS)]
                        n_add += 1
                        if eng == "pool":
                            nc.gpsimd.tensor_tensor(
                                half[:, js], pdst, cur.SP[:, js], op=add_op
                            )
                        else:
                            nc.vector.tensor_tensor(
                                half[:, js], pdst, cur.SP[:, js], op=add_op
                            )
                if not HALF_PS:
                    if pe_add:
                        evict(COPY_ENGS[n_copy % len(COPY_ENGS)], half, pmm[:])
                        n_copy += 1
                    else:
                        eng = ADD_ENGS[n_add % len(ADD_ENGS)]
                        n_add += 1
                        if eng == "pool":
                            nc.gpsimd.tensor_tensor(half, pmm[:], cur.SP[:], op=add_op)
                        else:
                            nc.vector.tensor_tensor(half, pmm[:], cur.SP[:], op=add_op)
                if nxt is not None:
                    emit_stage(nxt, it)
                if it % SG != SG - 1:
                    continue
                eng = STORE_ENGS[n_store % len(STORE_ENGS)]
                n_store += 1
                it0 = it - (SG - 1)
                dst = out[b, it0 * P : (it0 + SG) * P, :]
                if SG > 1:
                    eng.dma_start(dst.rearrange("(e q) d -> q e d", e=SG), ot[:])
                else:
                    eng.dma_start(dst, ot[:])

            if PROXY and b == bpc - 1:
                rb = consts.tile([P, 512], f16, tag="rb")
                nc.sync.dma_start(rb[:], out[b, :P, :512])
                nc.vector.tensor_add(sink[:], sink[:], rb[:])
            cur = nxt

        if PROXY:
            fin = consts.tile([P, P], f16, tag="fin")
            nc.vector.tensor_copy(fin[:], sink[:, :P])
            nc.sync.dma_start(outx[:], fin[:])

    nc.compile()
    return nc


def get_nc(bpc=BPC, repeats=1, tune=None):
    key = (bpc, repeats, str(sorted((tune or {}).items())))
    if key not in _CACHE:
        _CACHE[key] = _build(bpc, repeats, tune)
    return _CACHE[key]


def _make_runner(nc):
    """Reusable sharded-jit executor for the compiled Bass program."""
    import jax
    from jax.experimental.shard_map import shard_map
    from jax.sharding import Mesh, PartitionSpec

    from concourse import mybir
    from concourse.bass2jax import (
        _bass_exec_p,
        install_neuronx_cc_hook,
        partition_id_tensor,
    )

    install_neuronx_cc_hook()

    partition_name = nc.partition_id_tensor.name if nc.partition_id_tensor else None
    in_names, out_names, out_avals, zero_shapes = [], [], [], []
    for alloc in nc.m.functions[0].allocations:
        if not isinstance(alloc, mybir.MemoryLocationSet):
            continue
        name = alloc.memorylocations[0].name
        if alloc.kind == "ExternalInput":
            if name != partition_name:
                in_names.append(name)
        elif alloc.kind == "ExternalOutput":
            out_names.append(name)
            shape = tuple(alloc.tensor_shape)
            dtype = mybir.dt.np(alloc.dtype)
            out_avals.append(jax.core.ShapedArray(shape, dtype))
            zero_shapes.append((shape, dtype))
    n_params = len(in_names)
    n_outs = len(out_avals)
    all_in_names = list(in_names) + out_names
    if partition_name is not None:
        all_in_names.append(partition_name)
    donate = tuple(range(n_params, n_params + n_outs))

    def _body(*args):
        operands = list(args)
        if partition_name is not None:
            operands.append(partition_id_tensor())
        outs = _bass_exec_p.bind(
            *operands,
            out_avals=tuple(out_avals),
            in_names=tuple(all_in_names),
            out_names=tuple(out_names),
            lowering_input_output_aliases=(),
            sim_require_finite=True,
            sim_require_nnan=True,
            nc=nc,
        )
        return tuple(outs)

    devices = jax.devices()[:N_CORES]
    mesh = Mesh(np.asarray(devices), ("core",))
    in_specs = (PartitionSpec("core"),) * (n_params + n_outs)
    out_specs = (PartitionSpec("core"),) * len(out_names)
    sharded = jax.jit(
        shard_map(
            _body, mesh=mesh, in_specs=in_specs, out_specs=out_specs, check_rep=False
        ),
        donate_argnums=donate,
        keep_unused=True,
    )

    import jax.numpy as jnp
    from jax.sharding import NamedSharding

    zeros_fn = jax.jit(
        lambda: tuple(
            jnp.zeros((N_CORES * s[0], *s[1:]), dt) for s, dt in zero_shapes
        ),
        out_shardings=tuple(
            NamedSharding(mesh, PartitionSpec("core")) for _ in zero_shapes
        ),
    )

    def run(in_maps):
        concat_in = [
            np.concatenate(
                [np.asarray(in_maps[c][name]) for c in range(N_CORES)], axis=0
            )
            for name in in_names
        ]
        out_arrs = sharded(*concat_in, *zeros_fn())
        return [
            {
                name: np.asarray(out_arrs[i]).reshape(
                    N_CORES, *out_avals[i].shape
                )[c]
                for i, name in enumerate(out_names)
            }
            for c in range(N_CORES)
        ]

    return run


def kernel(batch_size=None, body=None, pun=None, w_u=None, **_):
    if "runner" not in _CACHE:
        _CACHE["runner"] = _make_runner(get_nc())
    # fp16 cast + [b, L, D] -> [b, D, L] layout transpose on the host
    bodyT = np.ascontiguousarray(
        np.asarray(body, dtype=np.float32).astype(np.float16).transpose(0, 2, 1)
    )
    punT = np.ascontiguousarray(
        np.asarray(pun, dtype=np.float32).astype(np.float16).transpose(0, 2, 1)
    )
    w_u = np.ascontiguousarray(w_u, dtype=np.float32).reshape(3 * D, 1)
    in_maps = [
        {
            "body": bodyT[c * BPC : (c + 1) * BPC],
            "pun": punT[c * BPC : (c + 1) * BPC],
            "w_u": w_u,
        }
        for c in range(N_CORES)
    ]
    results = _CACHE["runner"](in_maps)
    out16 = np.concatenate([results[c]["out"] for c in range(N_CORES)], axis=0)
    return out16.astype(np.float32)

